# revision 1
# baseline (speedup 1.0000x reference)
"""Trainium2 Bass kernel for a 3-layer GraphSAGE GNN (mean aggregation + BN + ReLU).

Self-contained: kernel(**inputs) -> np.ndarray [50000, 128] float32.

Strategy (8 NeuronCores, SPMD):
  - Nodes sharded 8 ways (6272/core = 49 tiles of 128). Edges assigned to the
    core owning their destination, sorted by dst, binned per 128-node tile.
  - Per-edge source features fetched with dma_gather from a per-core HBM table
    (bf16, 256B rows). Segment-sum done as one-hot matmuls on TensorE.
  - Dense part (agg@Wl + h@Wr + b) feature-major with constant stationary
    weights; BN stats via ACT accumulators + AllReduce; h redistributed with
    AllGather each layer.
"""
import sys, types
import numpy as np
import ml_dtypes

BF16 = ml_dtypes.bfloat16

# ---------------- problem constants (hardcoded per the task) ----------------
N = 50000
E = 800000
FIN = 3
H = 64
OUT = 128
NCORES = 8
P = 128                 # partitions / node tile
TILES = 49              # tiles per core
SH = TILES * P          # 6272 nodes per core (padded)
NTAB = NCORES * SH      # 50176 table rows
SPLIT = NTAB // 2       # 25088: lo rows [0, SPLIT), hi rows [SPLIT, NTAB)
CHUNK_TILES = 7         # tiles per gather chunk
NCHUNKS = TILES // CHUNK_TILES
EPS = 1e-5

_CACHE = {}
SKIP_COLLECTIVES = False


def _install_ntff_shim():
    import antenv
    if hasattr(antenv, "axon_hooks"):
        return
    mod = types.ModuleType("antenv.axon_hooks")
    _hook = [None]
    mod.set_axon_ntff_profile_hook = lambda h: _hook.__setitem__(0, h)
    mod.get_axon_ntff_profile_hook = lambda: _hook[0]
    sys.modules["antenv.axon_hooks"] = mod
    antenv.axon_hooks = mod
    try:
        from trn_agent_boot.trn_boot import _ntff_profile_via_ctypes
        h = _ntff_profile_via_ctypes("/opt/axon/libaxon_pjrt.so")
        if h is not None:
            mod.set_axon_ntff_profile_hook(h)
    except Exception:
        pass


# ---------------------------- host preprocessing ----------------------------

def _wrap_idx(arr):
    """int16 position-i -> partition i%16, col i//16; replicated to 128 parts."""
    n = arr.shape[0]
    assert n % 16 == 0
    base = arr.reshape(n // 16, 16).T.astype(np.int16)      # [16, n/16]
    return np.tile(base, (8, 1))                            # [128, n/16]


def _prep(edge_index):
    """Per-core gather/selection structures. Returns (B2L, B2H, per_core list)."""
    src = edge_index[0].astype(np.int64)
    dst = edge_index[1].astype(np.int64)

    deg = np.bincount(dst, minlength=N).astype(np.float64)
    invdeg = (1.0 / np.maximum(deg, 1.0)).astype(np.float32)        # [N]

    core = dst // SH                                   # owning core per edge
    tile = (dst % SH) // P                             # tile within core
    loc = dst % P                                      # dst offset within tile
    ishi = (src >= SPLIT).astype(np.int64)

    # group key: (core, tile, ishi); count per group to fix block counts
    key = (core * TILES + tile) * 2 + ishi
    counts = np.bincount(key, minlength=NCORES * TILES * 2)
    cnt_lo = counts[0::2].reshape(NCORES, TILES)
    cnt_hi = counts[1::2].reshape(NCORES, TILES)
    B2L = int(np.ceil(cnt_lo.max() / P))
    B2H = int(np.ceil(cnt_hi.max() / P))

    # stable ordering of edges by group
    order = np.argsort(key, kind="stable")
    ksort = key[order]
    starts = np.searchsorted(ksort, np.arange(NCORES * TILES * 2))
    ends = np.append(starts[1:], len(order))

    per_core = []
    for k in range(NCORES):
        nlo = TILES * B2L * P
        nhi = TILES * B2H * P
        idx_lo = np.zeros(nlo, np.int64)
        idx_hi = np.zeros(nhi, np.int64)
        sel_lo = np.full(nlo, -1.0, np.float32)
        sel_hi = np.full(nhi, -1.0, np.float32)
        for t in range(TILES):
            g = (k * TILES + t) * 2
            for hi in (0, 1):
                idxs = order[starts[g + hi]:ends[g + hi]]
                c = len(idxs)
                if hi:
                    base = t * B2H * P
                    idx_hi[base:base + c] = src[idxs] - SPLIT
                    sel_hi[base:base + c] = loc[idxs]
                else:
                    base = t * B2L * P
                    idx_lo[base:base + c] = src[idxs]
                    sel_lo[base:base + c] = loc[idxs]
        # slot i -> output partition i%128, block i//128 (dma_gather layout)
        d = dict(
            idx_lo=_wrap_idx(idx_lo.astype(np.int16)),
            idx_hi=_wrap_idx(idx_hi.astype(np.int16)),
            dsel_lo=np.ascontiguousarray(sel_lo.reshape(TILES * B2L, P).T),
            dsel_hi=np.ascontiguousarray(sel_hi.reshape(TILES * B2H, P).T),
        )
        # per-node data for this core
        lo_n, hi_n = k * SH, min((k + 1) * SH, N)
        iv = np.ones(SH, np.float32)
        iv[: hi_n - lo_n] = invdeg[lo_n:hi_n]
        d["invdegT"] = np.tile(iv[None, :], (H, 1)).astype(np.float32)  # [64, SH]
        om = np.zeros(SH, np.float32)
        om[: hi_n - lo_n] = 1.0
        d["onesmask"] = om[None, :].astype(BF16)                        # [1, SH]
        d["shard_len"] = hi_n - lo_n
        per_core.append(d)
    return B2L, B2H, per_core


# ------------------------------- bass program -------------------------------

def _build(B2L, B2H, dbg_layers=3, dbg_stage="full"):
    import concourse.bass as bass
    import concourse.bacc as bacc
    import concourse.tile as tile
    import concourse.mybir as mybir

    dt = mybir.dt
    Alu = mybir.AluOpType
    Act = mybir.ActivationFunctionType

    nc = bacc.Bacc("TRN2", target_bir_lowering=False, debug=False,
                   num_devices=NCORES)

    # ---------------- I/O ----------------
    def inp(name, shape, d):
        return nc.dram_tensor(name, list(shape), d, kind="ExternalInput")

    table0 = inp("table0", [NTAB, P], dt.bfloat16)
    xTown = inp("xTown", [FIN, SH], dt.bfloat16)
    idx_lo = inp("idx_lo", [P, TILES * B2L * 8], dt.int16)
    idx_hi = inp("idx_hi", [P, TILES * B2H * 8], dt.int16)
    dsel_lo = inp("dsel_lo", [P, TILES * B2L], dt.float32)
    dsel_hi = inp("dsel_hi", [P, TILES * B2H], dt.float32)
    invdegT = inp("invdegT", [H, SH], dt.float32)
    onesmask = inp("onesmask", [1, SH], dt.bfloat16)
    iota = inp("iota", [P, P], dt.bfloat16)
    identb = inp("identb", [H, H], dt.bfloat16)
    Wl0b = inp("Wl0b", [FIN, H], dt.bfloat16)
    Wl1b = inp("Wl1b", [H, H], dt.bfloat16)
    Wl2b = inp("Wl2b", [H, OUT], dt.bfloat16)
    Wr0b = inp("Wr0b", [FIN, H], dt.bfloat16)
    Wr1b = inp("Wr1b", [H, H], dt.bfloat16)
    Wr2b = inp("Wr2b", [H, OUT], dt.bfloat16)
    bl0b = inp("bl0b", [1, H], dt.bfloat16)
    bl1b = inp("bl1b", [1, H], dt.bfloat16)
    bl2b = inp("bl2b", [1, OUT], dt.bfloat16)
    g0c = inp("g0c", [H, 1], dt.float32)
    b0c = inp("b0c", [H, 1], dt.float32)
    g1c = inp("g1c", [H, 1], dt.float32)
    b1c = inp("b1c", [H, 1], dt.float32)

    out = nc.dram_tensor("out", [OUT, SH], dt.float32, kind="ExternalOutput")

    # DRAM scratch
    shard0 = nc.dram_tensor("shard0", [SH, P], dt.bfloat16)
    shard1 = nc.dram_tensor("shard1", [SH, P], dt.bfloat16)
    table1 = nc.dram_tensor("table1", [NTAB, P], dt.bfloat16, addr_space="Shared")
    table2 = nc.dram_tensor("table2", [NTAB, P], dt.bfloat16, addr_space="Shared")
    stats_in0 = nc.dram_tensor("stats_in0", [H, 2], dt.float32)
    stats_in1 = nc.dram_tensor("stats_in1", [H, 2], dt.float32)
    stats_out0 = nc.dram_tensor("stats_out0", [H, 2], dt.float32, addr_space="Shared")
    stats_out1 = nc.dram_tensor("stats_out1", [H, 2], dt.float32, addr_space="Shared")

    layers = [
        dict(table=table0, Wl=Wl0b, Wr=Wr0b, bl=bl0b, KA=FIN,
             g=g0c, b=b0c, HO=H, sin=stats_in0, sout=stats_out0,
             shard=shard0, tnext=table1),
        dict(table=table1, Wl=Wl1b, Wr=Wr1b, bl=bl1b, KA=H,
             g=g1c, b=b1c, HO=H, sin=stats_in1, sout=stats_out1,
             shard=shard1, tnext=table2),
        dict(table=table2, Wl=Wl2b, Wr=Wr2b, bl=bl2b, KA=H,
             g=None, b=None, HO=OUT, sin=None, sout=None,
             shard=None, tnext=None),
    ]

    NIL_C = CHUNK_TILES * B2L * P     # lo idxs per chunk
    NIH_C = CHUNK_TILES * B2H * P

    with tile.TileContext(nc) as tc:
        with tc.tile_pool(name="const", bufs=1) as cpool, \
             tc.tile_pool(name="work", bufs=3) as wpool, \
             tc.tile_pool(name="msgp", bufs=2) as mpool, \
             tc.tile_pool(name="spool", bufs=6) as spool, \
             tc.tile_pool(name="psum", bufs=2, space="PSUM") as pp:

            def load_const(t):
                sl = tuple(slice(0, s) for s in t.shape)
                tl = cpool.tile(list(t.shape), t.dtype, tag=t.name,
                                name=f"c_{t.name}")
                nc.sync.dma_start(out=tl[:], in_=t[sl])
                return tl

            xTown_t = load_const(xTown)
            idxlo_t = load_const(idx_lo)
            idxhi_t = load_const(idx_hi)
            dsello_t = load_const(dsel_lo)
            dselhi_t = load_const(dsel_hi)
            invdegT_t = load_const(invdegT)
            onesmask_t = load_const(onesmask)
            iota_t = load_const(iota)
            identb_t = load_const(identb)
            W_t = {l: tuple(load_const(t) for t in ts)
                   for l, ts in {0: (Wl0b, Wr0b, bl0b),
                                 1: (Wl1b, Wr1b, bl1b),
                                 2: (Wl2b, Wr2b, bl2b)}.items()}
            bn_t = {0: (load_const(g0c), load_const(b0c)),
                    1: (load_const(g1c), load_const(b1c))}

            # persistent regions
            yT_reg = cpool.tile([H, SH], dt.bfloat16, tag="yT_reg")
            hT = {1: cpool.tile([H, SH], dt.bfloat16, tag="hT1", name="hT1"),
                  2: cpool.tile([H, SH], dt.bfloat16, tag="hT2", name="hT2")}
            h_node = cpool.tile([P, TILES, P], dt.bfloat16, tag="h_node")
            nc.vector.memset(h_node[:], 0.0)
            ssum = cpool.tile([H, TILES], dt.float32, tag="ssum")
            ssq = cpool.tile([H, TILES], dt.float32, tag="ssq")

            own_t = {0: xTown_t, 1: hT[1], 2: hT[2]}

            if dbg_stage == "consts_only":
                dbg1 = wpool.tile([P, P], dt.float32, tag="dbg1")
                nc.vector.tensor_copy(out=dbg1[:], in_=iota_t[:])
                nc.sync.dma_start(out=out[0:P, 0:P], in_=dbg1[:])
                dbg2 = wpool.tile([H, SH], dt.float32, tag="dbg2")
                nc.vector.tensor_copy(out=dbg2[:], in_=invdegT_t[:])
                nc.sync.dma_start(out=out[0:H, 0:SH], in_=dbg2[:])

            for l, L in enumerate(layers):
                if dbg_stage == "consts_only":
                    break
                if l >= dbg_layers:
                    break
                table = L["table"]
                Wl_t, Wr_t, bl_t = W_t[l]
                KA = L["KA"]
                HO = L["HO"]
                own = own_t[l]
                for c in range(NCHUNKS):
                    msgL = mpool.tile([P, CHUNK_TILES * B2L, P], dt.bfloat16,
                                      tag="msgL")
                    msgH = mpool.tile([P, CHUNK_TILES * B2H, P], dt.bfloat16,
                                      tag="msgH")
                    if dbg_stage != "g_hi":
                        nc.gpsimd.dma_gather(
                            out_ap=msgL[:], in_ap=table[0:min(32768, NTAB), :],
                            idxs_ap=idxlo_t[:, c * (NIL_C // 16):(c + 1) * (NIL_C // 16)],
                            num_idxs=NIL_C, num_idxs_reg=NIL_C, elem_size=P,
                            single_packet=False)
                    else:
                        nc.vector.memset(msgL[:], 0.0)
                    if dbg_stage != "g_lo":
                        nc.gpsimd.dma_gather(
                            out_ap=msgH[:], in_ap=table[SPLIT:NTAB, :],
                            idxs_ap=idxhi_t[:, c * (NIH_C // 16):(c + 1) * (NIH_C // 16)],
                            num_idxs=NIH_C, num_idxs_reg=NIH_C, elem_size=P,
                            single_packet=False)
                    else:
                        nc.vector.memset(msgH[:], 0.0)
                    if dbg_stage in ("gather_only", "g_lo", "g_hi"):
                        dbgf = wpool.tile([P, P], dt.float32, tag="dbgf")
                        nc.vector.tensor_copy(out=dbgf[:], in_=msgL[:, 0, :])
                        nc.sync.dma_start(out=out[0:P, c * P:(c + 1) * P],
                                          in_=dbgf[:])
                        dbgh = wpool.tile([P, P], dt.float32, tag="dbgh")
                        nc.vector.tensor_copy(out=dbgh[:], in_=msgH[:, 0, :])
                        nc.sync.dma_start(out=out[0:P, (NCHUNKS + c) * P:(NCHUNKS + c + 1) * P],
                                          in_=dbgh[:])
                        continue
                    for tt in range(CHUNK_TILES):
                        t = c * CHUNK_TILES + tt
                        aggps = pp.tile([P, P], dt.float32, tag="aggps")
                        nb = B2L + B2H
                        for b in range(nb):
                            if b < B2L:
                                mblk = msgL[:, tt * B2L + b, 0:KA]
                                j = t * B2L + b
                                dcol = dsello_t[:, j:j + 1]
                            else:
                                bb = b - B2L
                                mblk = msgH[:, tt * B2H + bb, 0:KA]
                                j = t * B2H + bb
                                dcol = dselhi_t[:, j:j + 1]
                            S = spool.tile([P, P], dt.bfloat16, tag="S")
                            nc.vector.tensor_scalar(
                                out=S[:], in0=iota_t[:], scalar1=dcol,
                                scalar2=None, op0=Alu.is_equal)
                            nc.tensor.matmul(out=aggps[0:KA, :], lhsT=mblk,
                                             rhs=S[:],
                                             start=(b == 0), stop=(b == nb - 1))
                        # mean-scale + cast: aggb[f, n] (bf16)
                        aggb = wpool.tile([KA, P], dt.bfloat16,
                                          tag=f"aggb{KA}", name=f"aggb{KA}")
                        nc.vector.tensor_tensor(
                            out=aggb[:], in0=aggps[0:KA, :],
                            in1=invdegT_t[0:KA, t * P:(t + 1) * P], op=Alu.mult)
                        if dbg_stage == "agg":
                            continue
                        # dense: yT = Wl^T aggb + Wr^T own + bl^T mask
                        yps = pp.tile([HO, P], dt.float32, tag="yps")
                        nc.tensor.matmul(out=yps[:], lhsT=Wl_t[:], rhs=aggb[:],
                                         start=True, stop=False)
                        nc.tensor.matmul(out=yps[:], lhsT=Wr_t[:],
                                         rhs=own[:, t * P:(t + 1) * P],
                                         start=False, stop=False)
                        nc.tensor.matmul(out=yps[:], lhsT=bl_t[:],
                                         rhs=onesmask_t[:, t * P:(t + 1) * P],
                                         start=False, stop=True)
                        if l < 2:
                            nc.scalar.activation(
                                out=yT_reg[:, t * P:(t + 1) * P], in_=yps[:],
                                func=Act.Copy,
                                accum_out=ssum[:, t:t + 1])
                            sq = wpool.tile([H, P], dt.float32, tag="sq")
                            nc.scalar.activation(
                                out=sq[:], in_=yps[:], func=Act.Square,
                                accum_out=ssq[:, t:t + 1])
                        else:
                            y2 = wpool.tile([OUT, P], dt.float32, tag="y2")
                            nc.scalar.activation(out=y2[:], in_=yps[:],
                                                 func=Act.Copy)
                            nc.sync.dma_start(out=out[:, t * P:(t + 1) * P],
                                              in_=y2[:])

                if dbg_stage != "full":
                    continue
                if l < 2:
                    # ---- BN stats allreduce ----
                    stats = wpool.tile([H, 2], dt.float32, tag="stats")
                    nc.vector.tensor_reduce(out=stats[:, 0:1], in_=ssum[:],
                                            axis=mybir.AxisListType.X, op=Alu.add)
                    nc.vector.tensor_reduce(out=stats[:, 1:2], in_=ssq[:],
                                            axis=mybir.AxisListType.X, op=Alu.add)
                    nc.sync.dma_start(out=L["sin"][0:H, 0:2], in_=stats[:])
                    if SKIP_COLLECTIVES:
                        nc.sync.dma_start(out=L["sout"][0:H, 0:2],
                                          in_=L["sin"][0:H, 0:2])
                    else:
                        nc.gpsimd.collective_compute(
                            "AllReduce", Alu.add,
                            replica_groups=[list(range(NCORES))],
                            ins=[L["sin"].ap().opt()], outs=[L["sout"].ap().opt()])
                    sg = wpool.tile([H, 2], dt.float32, tag="sg")
                    nc.sync.dma_start(out=sg[:], in_=L["sout"][0:H, 0:2])
                    # s = g / sqrt(var+eps); t = b - mu*s
                    mu = wpool.tile([H, 1], dt.float32, tag="mu")
                    nc.vector.tensor_scalar(out=mu[:], in0=sg[:, 0:1],
                                            scalar1=1.0 / N, scalar2=None,
                                            op0=Alu.mult)
                    var = wpool.tile([H, 1], dt.float32, tag="var")
                    nc.vector.tensor_scalar(out=var[:], in0=sg[:, 1:2],
                                            scalar1=1.0 / N, scalar2=None,
                                            op0=Alu.mult)
                    mu2 = wpool.tile([H, 1], dt.float32, tag="mu2")
                    nc.vector.tensor_tensor(out=mu2[:], in0=mu[:], in1=mu[:],
                                            op=Alu.mult)
                    nc.vector.tensor_tensor(out=var[:], in0=var[:], in1=mu2[:],
                                            op=Alu.subtract)
                    nc.vector.tensor_scalar(out=var[:], in0=var[:],
                                            scalar1=float(EPS), scalar2=None,
                                            op0=Alu.add)
                    std = wpool.tile([H, 1], dt.float32, tag="std")
                    nc.scalar.activation(out=std[:], in_=var[:], func=Act.Sqrt)
                    istd = wpool.tile([H, 1], dt.float32, tag="istd")
                    nc.vector.reciprocal(out=istd[:], in_=std[:])
                    g_t, bb_t = bn_t[l]
                    s_col = wpool.tile([H, 1], dt.float32, tag="s_col")
                    nc.vector.tensor_tensor(out=s_col[:], in0=g_t[:], in1=istd[:],
                                            op=Alu.mult)
                    ms = wpool.tile([H, 1], dt.float32, tag="ms")
                    nc.vector.tensor_tensor(out=ms[:], in0=mu[:], in1=s_col[:],
                                            op=Alu.mult)
                    t_col = wpool.tile([H, 1], dt.float32, tag="t_col")
                    nc.vector.tensor_tensor(out=t_col[:], in0=bb_t[:], in1=ms[:],
                                            op=Alu.subtract)
                    # ---- BN apply + relu + transpose + table write ----
                    hT_l = hT[l + 1]
                    for t in range(TILES):
                        nc.scalar.activation(
                            out=hT_l[:, t * P:(t + 1) * P],
                            in_=yT_reg[:, t * P:(t + 1) * P],
                            func=Act.Relu, scale=s_col[:, 0:1],
                            bias=t_col[:, 0:1])
                        ptr = pp.tile([P, H], dt.bfloat16, tag="ps_tr")
                        nc.tensor.transpose(out=ptr[:],
                                            in_=hT_l[:, t * P:(t + 1) * P],
                                            identity=identb_t[:])
                        nc.scalar.activation(out=h_node[:, t, 0:H], in_=ptr[:],
                                             func=Act.Copy)
                    nc.sync.dma_start(
                        out=L["shard"].ap().rearrange("(t p) d -> p t d", p=P),
                        in_=h_node[:])
                    if SKIP_COLLECTIVES:
                        nc.sync.dma_start(out=L["tnext"][0:SH, 0:P],
                                          in_=L["shard"][0:SH, 0:P])
                    else:
                        nc.gpsimd.collective_compute(
                            "AllGather", Alu.bypass,
                            replica_groups=[list(range(NCORES))],
                            ins=[L["shard"].ap().opt()],
                            outs=[L["tnext"].ap().opt()])

    nc.compile()
    return nc


# --------------------------------- runner -----------------------------------

def _get_nc(B2L, B2H):
    key = (B2L, B2H)
    if key not in _CACHE:
        _CACHE[key] = _build(B2L, B2H)
    return _CACHE[key]


def make_in_maps(x, Wl0, bl0, Wr0, g0, b0, Wl1, bl1, Wr1, g1, b1,
                 Wl2, bl2, Wr2, per_core):
    x = np.asarray(x, np.float32)
    tab0 = np.zeros((NTAB, P), np.float32)
    tab0[:N, :FIN] = x
    tab0 = tab0.astype(BF16)
    xTfull = np.zeros((FIN, NTAB), np.float32)
    xTfull[:, :N] = x.T
    xTb = xTfull.astype(BF16)

    common = dict(
        table0=tab0,
        iota=np.tile(np.arange(P, dtype=np.float32), (P, 1)).astype(BF16),
        identb=np.eye(H, dtype=np.float32).astype(BF16),
        Wl0b=np.asarray(Wl0, np.float32).astype(BF16),
        Wl1b=np.asarray(Wl1, np.float32).astype(BF16),
        Wl2b=np.asarray(Wl2, np.float32).astype(BF16),
        Wr0b=np.asarray(Wr0, np.float32).astype(BF16),
        Wr1b=np.asarray(Wr1, np.float32).astype(BF16),
        Wr2b=np.asarray(Wr2, np.float32).astype(BF16),
        bl0b=np.asarray(bl0, np.float32).reshape(1, H).astype(BF16),
        bl1b=np.asarray(bl1, np.float32).reshape(1, H).astype(BF16),
        bl2b=np.asarray(bl2, np.float32).reshape(1, OUT).astype(BF16),
        g0c=np.ascontiguousarray(np.asarray(g0, np.float32).reshape(H, 1)),
        b0c=np.ascontiguousarray(np.asarray(b0, np.float32).reshape(H, 1)),
        g1c=np.ascontiguousarray(np.asarray(g1, np.float32).reshape(H, 1)),
        b1c=np.ascontiguousarray(np.asarray(b1, np.float32).reshape(H, 1)),
    )

    in_maps = []
    for k in range(NCORES):
        d = per_core[k]
        m = dict(common)
        m["xTown"] = np.ascontiguousarray(xTb[:, k * SH:(k + 1) * SH])
        for key in ("idx_lo", "idx_hi", "dsel_lo", "dsel_hi", "invdegT",
                    "onesmask"):
            m[key] = d[key]
        in_maps.append(m)
    return in_maps


def run(inputs, trace=False):
    """Build+run; returns (full_output, BassKernelResults)."""
    _install_ntff_shim()
    from concourse import bass_utils

    edge_index = np.asarray(inputs["edge_index"])
    B2L, B2H, per_core = _prep(edge_index)
    nc = _get_nc(B2L, B2H)
    in_maps = make_in_maps(
        inputs["x"], inputs["Wl0"], inputs["bl0"], inputs["Wr0"],
        inputs["g0"], inputs["b0"], inputs["Wl1"], inputs["bl1"],
        inputs["Wr1"], inputs["g1"], inputs["b1"], inputs["Wl2"],
        inputs["bl2"], inputs["Wr2"], per_core)
    res = bass_utils.run_bass_kernel_spmd(nc, in_maps,
                                          core_ids=list(range(NCORES)),
                                          trace=trace)
    parts = []
    for k in range(NCORES):
        n_k = per_core[k]["shard_len"]
        parts.append(res.results[k]["out"][:, :n_k].T)
    full = np.ascontiguousarray(np.concatenate(parts, axis=0),
                                dtype=np.float32)
    return full, res


def kernel(x, edge_index, Wl0, bl0, Wr0, g0, b0, Wl1, bl1, Wr1, g1, b1,
           Wl2, bl2, Wr2):
    full, _ = run(dict(x=x, edge_index=edge_index, Wl0=Wl0, bl0=bl0, Wr0=Wr0,
                       g0=g0, b0=b0, Wl1=Wl1, bl1=bl1, Wr1=Wr1, g1=g1, b1=b1,
                       Wl2=Wl2, bl2=bl2, Wr2=Wr2))
    return full



# revision 4
# speedup vs baseline: 1.7656x; 1.7656x over previous
"""Trainium2 Bass kernel for a 3-layer GraphSAGE GNN (mean aggregation + BN + ReLU).

Self-contained: kernel(**inputs) -> np.ndarray [50000, 128] float32.

Strategy (8 NeuronCores, SPMD):
  - Nodes sharded 8 ways (6272/core = 49 tiles of 128). Edges assigned to the
    core owning their destination, sorted by dst, binned per 128-node tile.
  - Per-edge source features fetched with dma_gather from a per-core HBM table
    (bf16, 256B rows) across 4 SWDGE queues (latency-bound path; queues
    scale throughput ~linearly). Segment-sum via one-hot matmuls on TensorE;
    the one-hot S matrices are built in bulk (one broadcast is_equal per
    7-tile chunk) instead of per 128-edge block.
  - Dense part (agg@Wl + h@Wr) feature-major with stationary weights; bias
    applied in the ACT stage; BN stats via ACT accumulators + AllReduce;
    h redistributed with AllGather each layer.
"""
import sys, types
import numpy as np
import ml_dtypes

BF16 = ml_dtypes.bfloat16

# ---------------- problem constants (hardcoded per the task) ----------------
N = 50000
E = 800000
FIN = 3
H = 64
OUT = 128
NCORES = 8
P = 128                 # partitions / node tile
TILES = 49              # tiles per core
SH = TILES * P          # 6272 nodes per core (padded)
NTAB = NCORES * SH      # 50176 table rows
SPLIT = NTAB // 2       # 25088: lo rows [0, SPLIT), hi rows [SPLIT, NTAB)
CHUNK_TILES = 7         # tiles per gather chunk
NCHUNKS = TILES // CHUNK_TILES
EPS = 1e-5
NQ = 4                  # SWDGE queues for gathers
SCRATCH = 32768         # dynamic DMA scratch (descriptor rings), bytes/partition

_CACHE = {}
SKIP_COLLECTIVES = False


def _install_ntff_shim():
    import antenv
    if hasattr(antenv, "axon_hooks"):
        return
    mod = types.ModuleType("antenv.axon_hooks")
    _hook = [None]
    mod.set_axon_ntff_profile_hook = lambda h: _hook.__setitem__(0, h)
    mod.get_axon_ntff_profile_hook = lambda: _hook[0]
    sys.modules["antenv.axon_hooks"] = mod
    antenv.axon_hooks = mod
    try:
        from trn_agent_boot.trn_boot import _ntff_profile_via_ctypes
        h = _ntff_profile_via_ctypes("/opt/axon/libaxon_pjrt.so")
        if h is not None:
            mod.set_axon_ntff_profile_hook(h)
    except Exception:
        pass


# ---------------------------- host preprocessing ----------------------------

def _wrap_idx(arr):
    """int16 position-i -> partition i%16, col i//16; replicated to 128 parts."""
    n = arr.shape[0]
    assert n % 16 == 0
    base = arr.reshape(n // 16, 16).T.astype(np.int16)      # [16, n/16]
    return np.tile(base, (8, 1))                            # [128, n/16]


def _prep(edge_index):
    """Per-core gather/selection structures. Returns (B2L, B2H, per_core list)."""
    src = edge_index[0].astype(np.int64)
    dst = edge_index[1].astype(np.int64)

    deg = np.bincount(dst, minlength=N).astype(np.float64)
    invdeg = (1.0 / np.maximum(deg, 1.0)).astype(np.float32)        # [N]

    core = dst // SH                                   # owning core per edge
    tile = (dst % SH) // P                             # tile within core
    loc = dst % P                                      # dst offset within tile
    ishi = (src >= SPLIT).astype(np.int64)

    # group key: (core, tile, ishi); count per group to fix block counts
    key = (core * TILES + tile) * 2 + ishi
    counts = np.bincount(key, minlength=NCORES * TILES * 2)
    cnt_lo = counts[0::2].reshape(NCORES, TILES)
    cnt_hi = counts[1::2].reshape(NCORES, TILES)
    B2L = int(np.ceil(cnt_lo.max() / P))
    B2H = int(np.ceil(cnt_hi.max() / P))
    NB = B2L + B2H

    # stable ordering of edges by group
    order = np.argsort(key, kind="stable")
    ksort = key[order]
    starts = np.searchsorted(ksort, np.arange(NCORES * TILES * 2))
    ends = np.append(starts[1:], len(order))

    per_core = []
    for k in range(NCORES):
        nlo = TILES * B2L * P
        nhi = TILES * B2H * P
        idx_lo = np.zeros(nlo, np.int64)
        idx_hi = np.zeros(nhi, np.int64)
        sel_lo = np.full(nlo, -1.0, np.float32)
        sel_hi = np.full(nhi, -1.0, np.float32)
        for t in range(TILES):
            g = (k * TILES + t) * 2
            for hi in (0, 1):
                idxs = order[starts[g + hi]:ends[g + hi]]
                c = len(idxs)
                if hi:
                    base = t * B2H * P
                    idx_hi[base:base + c] = src[idxs] - SPLIT
                    sel_hi[base:base + c] = loc[idxs]
                else:
                    base = t * B2L * P
                    idx_lo[base:base + c] = src[idxs]
                    sel_lo[base:base + c] = loc[idxs]
        # slot i -> output partition i%128, block i//128 (dma_gather layout)
        slo = sel_lo.reshape(TILES * B2L, P).T              # [P, TILES*B2L]
        shi = sel_hi.reshape(TILES * B2H, P).T              # [P, TILES*B2H]
        dselc = np.empty((P, TILES * NB), np.float32)
        for t in range(TILES):
            dselc[:, t * NB:t * NB + B2L] = slo[:, t * B2L:(t + 1) * B2L]
            dselc[:, t * NB + B2L:(t + 1) * NB] = shi[:, t * B2H:(t + 1) * B2H]
        d = dict(
            idx_lo=_wrap_idx(idx_lo.astype(np.int16)),
            idx_hi=_wrap_idx(idx_hi.astype(np.int16)),
            dselc=np.ascontiguousarray(dselc.astype(BF16)),
        )
        # per-node data for this core
        lo_n, hi_n = k * SH, min((k + 1) * SH, N)
        iv = np.ones(SH, np.float32)
        iv[: hi_n - lo_n] = invdeg[lo_n:hi_n]
        d["invdegT"] = np.tile(iv[None, :], (H, 1)).astype(BF16)    # [64, SH]
        d["shard_len"] = hi_n - lo_n
        per_core.append(d)
    return B2L, B2H, per_core


# ------------------------------- bass program -------------------------------

def _build(B2L, B2H):
    import concourse.bass as bass
    import concourse.bacc as bacc
    import concourse.tile as tile
    import concourse.mybir as mybir

    dt = mybir.dt
    Alu = mybir.AluOpType
    Act = mybir.ActivationFunctionType

    nc = bacc.Bacc("TRN2", target_bir_lowering=False, debug=False,
                   num_devices=NCORES, num_swdge_queues=NQ,
                   dynamic_dma_scratch_size=SCRATCH)

    NB = B2L + B2H

    # ---------------- I/O ----------------
    def inp(name, shape, d):
        return nc.dram_tensor(name, list(shape), d, kind="ExternalInput")

    table0 = inp("table0", [NTAB, P], dt.bfloat16)
    xTown = inp("xTown", [FIN, SH], dt.bfloat16)
    idx_lo = inp("idx_lo", [P, TILES * B2L * 8], dt.int16)
    idx_hi = inp("idx_hi", [P, TILES * B2H * 8], dt.int16)
    dselc = inp("dselc", [P, TILES * NB], dt.bfloat16)
    invdegT = inp("invdegT", [H, SH], dt.bfloat16)
    iota = inp("iota", [P, P], dt.bfloat16)
    identb = inp("identb", [H, H], dt.bfloat16)
    Wl0b = inp("Wl0b", [FIN, H], dt.bfloat16)
    Wl1b = inp("Wl1b", [H, H], dt.bfloat16)
    Wl2b = inp("Wl2b", [H, OUT], dt.bfloat16)
    Wr0b = inp("Wr0b", [FIN, H], dt.bfloat16)
    Wr1b = inp("Wr1b", [H, H], dt.bfloat16)
    Wr2b = inp("Wr2b", [H, OUT], dt.bfloat16)
    bl0c = inp("bl0c", [H, 1], dt.float32)
    bl1c = inp("bl1c", [H, 1], dt.float32)
    bl2c = inp("bl2c", [OUT, 1], dt.float32)
    g0c = inp("g0c", [H, 1], dt.float32)
    b0c = inp("b0c", [H, 1], dt.float32)
    g1c = inp("g1c", [H, 1], dt.float32)
    b1c = inp("b1c", [H, 1], dt.float32)

    out = nc.dram_tensor("out", [OUT, SH], dt.float32, kind="ExternalOutput")

    # DRAM scratch
    shard0 = nc.dram_tensor("shard0", [SH, P], dt.bfloat16)
    shard1 = nc.dram_tensor("shard1", [SH, P], dt.bfloat16)
    table1 = nc.dram_tensor("table1", [NTAB, P], dt.bfloat16, addr_space="Shared")
    table2 = nc.dram_tensor("table2", [NTAB, P], dt.bfloat16, addr_space="Shared")
    stats_in0 = nc.dram_tensor("stats_in0", [H, 2], dt.float32)
    stats_in1 = nc.dram_tensor("stats_in1", [H, 2], dt.float32)
    stats_out0 = nc.dram_tensor("stats_out0", [H, 2], dt.float32, addr_space="Shared")
    stats_out1 = nc.dram_tensor("stats_out1", [H, 2], dt.float32, addr_space="Shared")

    layers = [
        dict(table=table0, Wl=Wl0b, Wr=Wr0b, bl=bl0c, KA=FIN,
             g=g0c, b=b0c, HO=H, sin=stats_in0, sout=stats_out0,
             shard=shard0, tnext=table1),
        dict(table=table1, Wl=Wl1b, Wr=Wr1b, bl=bl1c, KA=H,
             g=g1c, b=b1c, HO=H, sin=stats_in1, sout=stats_out1,
             shard=shard1, tnext=table2),
        dict(table=table2, Wl=Wl2b, Wr=Wr2b, bl=bl2c, KA=H,
             g=None, b=None, HO=OUT, sin=None, sout=None,
             shard=None, tnext=None),
    ]

    NIL_C = CHUNK_TILES * B2L * P     # lo idxs per chunk
    NIH_C = CHUNK_TILES * B2H * P
    NBLK_C = CHUNK_TILES * NB         # one-hot blocks per chunk

    with tile.TileContext(nc) as tc:
        with tc.tile_pool(name="const", bufs=1) as cpool, \
             tc.tile_pool(name="work", bufs=3) as wpool, \
             tc.tile_pool(name="msgp", bufs=2) as mpool, \
             tc.tile_pool(name="spool", bufs=1) as spool, \
             tc.tile_pool(name="psum", bufs=2, space="PSUM") as pp:

            def load_const(t):
                sl = tuple(slice(0, s) for s in t.shape)
                tl = cpool.tile(list(t.shape), t.dtype, tag=t.name,
                                name=f"c_{t.name}")
                nc.sync.dma_start(out=tl[:], in_=t[sl])
                return tl

            xTown_t = load_const(xTown)
            idxlo_t = load_const(idx_lo)
            idxhi_t = load_const(idx_hi)
            dselc_t = load_const(dselc)
            invdegT_t = load_const(invdegT)
            iota_t = load_const(iota)
            identb_t = load_const(identb)
            W_t = {l: tuple(load_const(t) for t in ts)
                   for l, ts in {0: (Wl0b, Wr0b, bl0c),
                                 1: (Wl1b, Wr1b, bl1c),
                                 2: (Wl2b, Wr2b, bl2c)}.items()}
            bn_t = {0: (load_const(g0c), load_const(b0c)),
                    1: (load_const(g1c), load_const(b1c))}

            # persistent regions
            yT_reg = cpool.tile([H, SH], dt.bfloat16, tag="yT_reg")
            hT = cpool.tile([H, SH], dt.bfloat16, tag="hT", name="hT")
            h_node = cpool.tile([P, TILES, P], dt.bfloat16, tag="h_node")
            nc.vector.memset(h_node[:], 0.0)
            ssum = cpool.tile([H, TILES], dt.float32, tag="ssum")
            ssq = cpool.tile([H, TILES], dt.float32, tag="ssq")

            own_t = {0: xTown_t, 1: hT, 2: hT}

            for l, L in enumerate(layers):
                table = L["table"]
                Wl_t, Wr_t, bl_t = W_t[l]
                KA = L["KA"]
                HO = L["HO"]
                own = own_t[l]
                for c in range(NCHUNKS):
                    msgL = mpool.tile([P, CHUNK_TILES * B2L, P], dt.bfloat16,
                                      tag="msgL")
                    msgH = mpool.tile([P, CHUNK_TILES * B2H, P], dt.bfloat16,
                                      tag="msgH")
                    nc.gpsimd.dma_gather(
                        out_ap=msgL[:], in_ap=table[0:SPLIT, :],
                        idxs_ap=idxlo_t[:, c * (NIL_C // 16):(c + 1) * (NIL_C // 16)],
                        num_idxs=NIL_C, num_idxs_reg=NIL_C, elem_size=P,
                        single_packet=False, queue_num=(2 * c) % NQ)
                    nc.gpsimd.dma_gather(
                        out_ap=msgH[:], in_ap=table[SPLIT:NTAB, :],
                        idxs_ap=idxhi_t[:, c * (NIH_C // 16):(c + 1) * (NIH_C // 16)],
                        num_idxs=NIH_C, num_idxs_reg=NIH_C, elem_size=P,
                        single_packet=False, queue_num=(2 * c + 1) % NQ)
                    # bulk one-hot build for the whole chunk:
                    #   S[p, j, c] = (iota[p, c] == dselc[p, chunk j])
                    S = spool.tile([P, NBLK_C, P], dt.bfloat16, tag="S")
                    i_bc = iota_t[:, :].unsqueeze(1).broadcast_to(
                        [P, NBLK_C, P])
                    d_bc = dselc_t[:, c * NBLK_C:(c + 1) * NBLK_C].unsqueeze(
                        2).broadcast_to([P, NBLK_C, P])
                    nc.vector.tensor_tensor(out=S[:], in0=i_bc, in1=d_bc,
                                            op=Alu.is_equal)
                    for tt in range(CHUNK_TILES):
                        t = c * CHUNK_TILES + tt
                        aggps = pp.tile([P, P], dt.float32, tag="aggps")
                        for b in range(NB):
                            if b < B2L:
                                mblk = msgL[:, tt * B2L + b, 0:KA]
                            else:
                                mblk = msgH[:, tt * B2H + (b - B2L), 0:KA]
                            nc.tensor.matmul(out=aggps[0:KA, :], lhsT=mblk,
                                             rhs=S[:, tt * NB + b, :],
                                             start=(b == 0), stop=(b == NB - 1))
                        # mean-scale + cast: aggb[f, n] (bf16)
                        aggb = wpool.tile([KA, P], dt.bfloat16,
                                          tag=f"aggb{KA}", name=f"aggb{KA}")
                        nc.vector.tensor_tensor(
                            out=aggb[:], in0=aggps[0:KA, :],
                            in1=invdegT_t[0:KA, t * P:(t + 1) * P], op=Alu.mult)
                        # dense: yT = Wl^T aggb + Wr^T own  (+ bias in ACT)
                        yps = pp.tile([HO, P], dt.float32, tag="yps")
                        nc.tensor.matmul(out=yps[:], lhsT=Wl_t[:], rhs=aggb[:],
                                         start=True, stop=False)
                        nc.tensor.matmul(out=yps[:], lhsT=Wr_t[:],
                                         rhs=own[:, t * P:(t + 1) * P],
                                         start=False, stop=True)
                        if l < 2:
                            nc.scalar.activation(
                                out=yT_reg[:, t * P:(t + 1) * P], in_=yps[:],
                                func=Act.Identity, bias=bl_t[:, 0:1],
                                accum_out=ssum[:, t:t + 1])
                            sq = wpool.tile([H, P], dt.float32, tag="sq")
                            nc.scalar.activation(
                                out=sq[:], in_=yps[:], func=Act.Square,
                                bias=bl_t[:, 0:1],
                                accum_out=ssq[:, t:t + 1])
                        else:
                            y2 = wpool.tile([OUT, P], dt.float32, tag="y2")
                            nc.scalar.activation(out=y2[:], in_=yps[:],
                                                 func=Act.Identity,
                                                 bias=bl_t[:, 0:1])
                            nc.sync.dma_start(out=out[:, t * P:(t + 1) * P],
                                              in_=y2[:])

                if l < 2:
                    # ---- BN stats allreduce ----
                    stats = wpool.tile([H, 2], dt.float32, tag="stats")
                    nc.vector.tensor_reduce(out=stats[:, 0:1], in_=ssum[:],
                                            axis=mybir.AxisListType.X, op=Alu.add)
                    nc.vector.tensor_reduce(out=stats[:, 1:2], in_=ssq[:],
                                            axis=mybir.AxisListType.X, op=Alu.add)
                    nc.sync.dma_start(out=L["sin"][0:H, 0:2], in_=stats[:])
                    if SKIP_COLLECTIVES:
                        nc.sync.dma_start(out=L["sout"][0:H, 0:2],
                                          in_=L["sin"][0:H, 0:2])
                    else:
                        nc.gpsimd.collective_compute(
                            "AllReduce", Alu.add,
                            replica_groups=[list(range(NCORES))],
                            ins=[L["sin"].ap().opt()], outs=[L["sout"].ap().opt()])
                    sg = wpool.tile([H, 2], dt.float32, tag="sg")
                    nc.sync.dma_start(out=sg[:], in_=L["sout"][0:H, 0:2])
                    # s = g / sqrt(var+eps); t = b - mu*s
                    mu = wpool.tile([H, 1], dt.float32, tag="mu")
                    nc.vector.tensor_scalar(out=mu[:], in0=sg[:, 0:1],
                                            scalar1=1.0 / N, scalar2=None,
                                            op0=Alu.mult)
                    var = wpool.tile([H, 1], dt.float32, tag="var")
                    nc.vector.tensor_scalar(out=var[:], in0=sg[:, 1:2],
                                            scalar1=1.0 / N, scalar2=None,
                                            op0=Alu.mult)
                    mu2 = wpool.tile([H, 1], dt.float32, tag="mu2")
                    nc.vector.tensor_tensor(out=mu2[:], in0=mu[:], in1=mu[:],
                                            op=Alu.mult)
                    nc.vector.tensor_tensor(out=var[:], in0=var[:], in1=mu2[:],
                                            op=Alu.subtract)
                    nc.vector.tensor_scalar(out=var[:], in0=var[:],
                                            scalar1=float(EPS), scalar2=None,
                                            op0=Alu.add)
                    std = wpool.tile([H, 1], dt.float32, tag="std")
                    nc.scalar.activation(out=std[:], in_=var[:], func=Act.Sqrt)
                    istd = wpool.tile([H, 1], dt.float32, tag="istd")
                    nc.vector.reciprocal(out=istd[:], in_=std[:])
                    g_t, bb_t = bn_t[l]
                    s_col = wpool.tile([H, 1], dt.float32, tag="s_col")
                    nc.vector.tensor_tensor(out=s_col[:], in0=g_t[:], in1=istd[:],
                                            op=Alu.mult)
                    ms = wpool.tile([H, 1], dt.float32, tag="ms")
                    nc.vector.tensor_tensor(out=ms[:], in0=mu[:], in1=s_col[:],
                                            op=Alu.mult)
                    t_col = wpool.tile([H, 1], dt.float32, tag="t_col")
                    nc.vector.tensor_tensor(out=t_col[:], in0=bb_t[:], in1=ms[:],
                                            op=Alu.subtract)
                    # ---- BN apply + relu + transpose + table write ----
                    for t in range(TILES):
                        nc.scalar.activation(
                            out=hT[:, t * P:(t + 1) * P],
                            in_=yT_reg[:, t * P:(t + 1) * P],
                            func=Act.Relu, scale=s_col[:, 0:1],
                            bias=t_col[:, 0:1])
                        ptr = pp.tile([P, H], dt.bfloat16, tag="ps_tr")
                        nc.tensor.transpose(out=ptr[:],
                                            in_=hT[:, t * P:(t + 1) * P],
                                            identity=identb_t[:])
                        nc.scalar.activation(out=h_node[:, t, 0:H], in_=ptr[:],
                                             func=Act.Copy)
                    nc.sync.dma_start(
                        out=L["shard"].ap().rearrange("(t p) d -> p t d", p=P),
                        in_=h_node[:])
                    if SKIP_COLLECTIVES:
                        nc.sync.dma_start(out=L["tnext"][0:SH, 0:P],
                                          in_=L["shard"][0:SH, 0:P])
                    else:
                        nc.gpsimd.collective_compute(
                            "AllGather", Alu.bypass,
                            replica_groups=[list(range(NCORES))],
                            ins=[L["shard"].ap().opt()],
                            outs=[L["tnext"].ap().opt()])

    nc.compile()
    return nc


# --------------------------------- runner -----------------------------------

def _get_nc(B2L, B2H):
    key = (B2L, B2H)
    if key not in _CACHE:
        _CACHE[key] = _build(B2L, B2H)
    return _CACHE[key]


def make_in_maps(x, Wl0, bl0, Wr0, g0, b0, Wl1, bl1, Wr1, g1, b1,
                 Wl2, bl2, Wr2, per_core):
    x = np.asarray(x, np.float32)
    tab0 = np.zeros((NTAB, P), np.float32)
    tab0[:N, :FIN] = x
    tab0 = tab0.astype(BF16)
    xTfull = np.zeros((FIN, NTAB), np.float32)
    xTfull[:, :N] = x.T
    xTb = xTfull.astype(BF16)

    common = dict(
        table0=tab0,
        iota=np.tile(np.arange(P, dtype=np.float32), (P, 1)).astype(BF16),
        identb=np.eye(H, dtype=np.float32).astype(BF16),
        Wl0b=np.asarray(Wl0, np.float32).astype(BF16),
        Wl1b=np.asarray(Wl1, np.float32).astype(BF16),
        Wl2b=np.asarray(Wl2, np.float32).astype(BF16),
        Wr0b=np.asarray(Wr0, np.float32).astype(BF16),
        Wr1b=np.asarray(Wr1, np.float32).astype(BF16),
        Wr2b=np.asarray(Wr2, np.float32).astype(BF16),
        bl0c=np.ascontiguousarray(np.asarray(bl0, np.float32).reshape(H, 1)),
        bl1c=np.ascontiguousarray(np.asarray(bl1, np.float32).reshape(H, 1)),
        bl2c=np.ascontiguousarray(np.asarray(bl2, np.float32).reshape(OUT, 1)),
        g0c=np.ascontiguousarray(np.asarray(g0, np.float32).reshape(H, 1)),
        b0c=np.ascontiguousarray(np.asarray(b0, np.float32).reshape(H, 1)),
        g1c=np.ascontiguousarray(np.asarray(g1, np.float32).reshape(H, 1)),
        b1c=np.ascontiguousarray(np.asarray(b1, np.float32).reshape(H, 1)),
    )

    in_maps = []
    for k in range(NCORES):
        d = per_core[k]
        m = dict(common)
        m["xTown"] = np.ascontiguousarray(xTb[:, k * SH:(k + 1) * SH])
        for key in ("idx_lo", "idx_hi", "dselc", "invdegT"):
            m[key] = d[key]
        in_maps.append(m)
    return in_maps


def run(inputs, trace=False):
    """Build+run; returns (full_output, BassKernelResults)."""
    _install_ntff_shim()
    from concourse import bass_utils

    edge_index = np.asarray(inputs["edge_index"])
    B2L, B2H, per_core = _prep(edge_index)
    nc = _get_nc(B2L, B2H)
    in_maps = make_in_maps(
        inputs["x"], inputs["Wl0"], inputs["bl0"], inputs["Wr0"],
        inputs["g0"], inputs["b0"], inputs["Wl1"], inputs["bl1"],
        inputs["Wr1"], inputs["g1"], inputs["b1"], inputs["Wl2"],
        inputs["bl2"], inputs["Wr2"], per_core)
    res = bass_utils.run_bass_kernel_spmd(nc, in_maps,
                                          core_ids=list(range(NCORES)),
                                          trace=trace)
    parts = []
    for k in range(NCORES):
        n_k = per_core[k]["shard_len"]
        parts.append(res.results[k]["out"][:, :n_k].T)
    full = np.ascontiguousarray(np.concatenate(parts, axis=0),
                                dtype=np.float32)
    return full, res


def kernel(x, edge_index, Wl0, bl0, Wr0, g0, b0, Wl1, bl1, Wr1, g1, b1,
           Wl2, bl2, Wr2):
    full, _ = run(dict(x=x, edge_index=edge_index, Wl0=Wl0, bl0=bl0, Wr0=Wr0,
                       g0=g0, b0=b0, Wl1=Wl1, bl1=bl1, Wr1=Wr1, g1=g1, b1=b1,
                       Wl2=Wl2, bl2=bl2, Wr2=Wr2))
    return full
